# revision 1
# baseline (speedup 1.0000x reference)
"""Causal self-attention (B=2, N=2048, D=768, H=12, HD=64) on 8 TRN2 NeuronCores.

Sharding: tensor-parallel over (batch, head). Core c handles batch b = c//4 and
heads [3*(c%4), 3*(c%4)+3). Each core computes its 3 heads' attention plus the
matching 192 columns of the output projection (row-parallel W_proj), returning a
partial [2048, 768] output. Host sums the 4 partials per batch element and adds
b_proj.

Per-core kernel layout (all fp32):
  - x arrives transposed (xT [768, 2048]) so the KQV projection produces q/k
    directly in [head_dim, token] orientation (q_T, k_T [64, 2048]).
  - scores are computed pre-transposed, S_T[k, q] = k_T^T-slice @ q_T, so the
    softmax denominator is a matmul reduction: v is padded with a ones column
    and P_T = exp(S_T/8) feeds sa_T[d, q] / denom[q] in one accumulation chain.
  - causal masking is multiplicative on P_T with 4 static [128, 512] masks
    (only diagonal blocks need it; fully-masked blocks are skipped).
  - sa_T [64, 2048] per head is exactly the lhsT the projection needs.
"""

import numpy as np

import concourse.bass as bass
import concourse.mybir as mybir
import concourse.tile as tile
from concourse import bacc
from concourse.bass_utils import run_bass_kernel_spmd

F32 = mybir.dt.float32
AF = mybir.ActivationFunctionType

B, N, D = 2, 2048, 768
H, HD = 12, 64
HEADS_PER_CORE = 3
N_CORES = 8
NT = N // 128          # 16 token tiles of 128
NS = N // 512          # 4 query spans of 512
DC = D // 128          # 6 contraction chunks of 128

# Matmul operand dtype knob: float32 (exact, 4 cycles/row) or float32r
# (1 cycle/row at free-dim >= 256, reduced precision on HW).
MM_DT = mybir.dt.float32

_CACHE = {}


def _mm_ap(ap):
    if MM_DT == F32:
        return ap
    return ap.bitcast(MM_DT)


def build():
    nc = bacc.Bacc("TRN2", target_bir_lowering=False, debug=False)

    xT_d = nc.dram_tensor("xT", [D, N], F32, kind="ExternalInput").ap()
    wqk_d = nc.dram_tensor("wqk", [128, HEADS_PER_CORE, DC, 128], F32, kind="ExternalInput").ap()
    wv_d = nc.dram_tensor("wv", [128, DC, 256], F32, kind="ExternalInput").ap()
    wp_d = nc.dram_tensor("wp", [64, HEADS_PER_CORE, D], F32, kind="ExternalInput").ap()
    bqk_d = nc.dram_tensor("bqk", [128, HEADS_PER_CORE], F32, kind="ExternalInput").ap()
    bvb_d = nc.dram_tensor("bvb", [128, 192], F32, kind="ExternalInput").ap()
    mask_d = nc.dram_tensor("masks", [128, 4, 512], F32, kind="ExternalInput").ap()
    ones_d = nc.dram_tensor("ones64", [1, 64], F32, kind="ExternalInput").ap()
    out_d = nc.dram_tensor("out", [N, D], F32, kind="ExternalOutput").ap()

    with tile.TileContext(nc) as tc:
        with tc.tile_pool(name="cn", bufs=1) as cn, \
             tc.tile_pool(name="qk", bufs=2) as qkp, \
             tc.tile_pool(name="pt", bufs=4) as ptp, \
             tc.tile_pool(name="sm", bufs=2) as smp, \
             tc.tile_pool(name="ot", bufs=3) as otp, \
             tc.tile_pool(name="psS", bufs=2, space="PSUM") as psS, \
             tc.tile_pool(name="psP", bufs=4, space="PSUM") as psP, \
             tc.tile_pool(name="psM", bufs=2, space="PSUM") as psM:

            # ---- constant loads ----
            x_sb = cn.tile([128, DC, N], F32, name="x_sb")
            nc.sync.dma_start(x_sb[:], xT_d.rearrange("(c p) n -> p c n", p=128))
            wqk_sb = cn.tile([128, HEADS_PER_CORE, DC, 128], F32, name="wqk_sb")
            nc.sync.dma_start(wqk_sb[:], wqk_d)
            wv_sb = cn.tile([128, DC, 256], F32, name="wv_sb")
            nc.sync.dma_start(wv_sb[:], wv_d)
            wp_sb = cn.tile([64, HEADS_PER_CORE, D], F32, name="wp_sb")
            nc.sync.dma_start(wp_sb[:], wp_d)
            bqk_sb = cn.tile([128, HEADS_PER_CORE], F32, name="bqk_sb")
            nc.sync.dma_start(bqk_sb[:], bqk_d)
            bvb_sb = cn.tile([128, 192], F32, name="bvb_sb")
            nc.sync.dma_start(bvb_sb[:], bvb_d)
            mask_sb = cn.tile([128, 4, 512], F32, name="mask_sb")
            nc.sync.dma_start(mask_sb[:], mask_d)
            ones_sb = cn.tile([1, 64], F32, name="ones_sb")
            nc.sync.dma_start(ones_sb[:], ones_d)

            vf = cn.tile([128, NT, HEADS_PER_CORE, 65], F32, name="vf")
            saT = cn.tile([64, HEADS_PER_CORE, N], F32, name="saT")

            # ---- V projection for all 3 heads fused: v[n, o] o in [0, 192) ----
            for nt in range(NT):
                v_ps = psM.tile([128, 512], F32, name="v_ps", tag="misc")
                for c in range(DC):
                    nc.tensor.matmul(
                        v_ps[:, 0:256],
                        _mm_ap(x_sb[:, c, nt * 128:(nt + 1) * 128]),
                        _mm_ap(wv_sb[:, c, :]),
                        start=(c == 0), stop=(c == DC - 1),
                    )
                nc.vector.tensor_tensor(
                    vf[:, nt, :, 0:64],
                    v_ps[:, 0:192].rearrange("p (h d) -> p h d", h=3),
                    bvb_sb[:].rearrange("p (h d) -> p h d", h=3),
                    op=mybir.AluOpType.add,
                )
                nc.vector.memset(vf[:, nt, :, 64:65], 1.0)

            # ---- per-head attention ----
            for j in range(HEADS_PER_CORE):
                qt = qkp.tile([64, N], F32, name="qt")
                kt = qkp.tile([64, N], F32, name="kt")
                for s in range(NS):
                    qk_ps = psM.tile([128, 512], F32, name="qk_ps", tag="misc")
                    for c in range(DC):
                        nc.tensor.matmul(
                            qk_ps[:],
                            _mm_ap(wqk_sb[:, j, c, :]),
                            _mm_ap(x_sb[:, c, s * 512:(s + 1) * 512]),
                            start=(c == 0), stop=(c == DC - 1),
                        )
                    nc.scalar.activation(qt[:, s * 512:(s + 1) * 512], qk_ps[0:64, :],
                                         AF.Identity, bias=bqk_sb[0:64, j:j + 1])
                    nc.scalar.activation(kt[:, s * 512:(s + 1) * 512], qk_ps[64:128, :],
                                         AF.Identity, bias=bqk_sb[64:128, j:j + 1])

                for s in range(NS):
                    nkt = 4 * s + 4
                    pv_ps = psP.tile([65, 512], F32, name="pv_ps", tag="pv")
                    for ktile in range(nkt):
                        sc_ps = psS.tile([128, 512], F32, name="sc_ps", tag="sc")
                        nc.tensor.matmul(
                            sc_ps[:],
                            _mm_ap(kt[:, ktile * 128:(ktile + 1) * 128]),
                            _mm_ap(qt[:, s * 512:(s + 1) * 512]),
                            start=True, stop=True,
                        )
                        pt = ptp.tile([128, 512], F32, name="pt")
                        nc.scalar.activation(pt[:], sc_ps[:], AF.Exp, scale=0.125)
                        if ktile >= 4 * s:
                            nc.vector.tensor_mul(pt[:], pt[:], mask_sb[:, ktile - 4 * s, :])
                        nc.tensor.matmul(
                            pv_ps[:],
                            _mm_ap(vf[:, ktile, j, :]),
                            _mm_ap(pt[:]),
                            start=(ktile == 0), stop=(ktile == nkt - 1),
                        )
                    rc = smp.tile([1, 512], F32, name="rc")
                    nc.vector.reciprocal(rc[:], pv_ps[64:65, :])
                    rb_ps = psM.tile([128, 512], F32, name="rb_ps", tag="misc")
                    nc.tensor.matmul(rb_ps[0:64, :], _mm_ap(ones_sb[:]), _mm_ap(rc[:]),
                                     start=True, stop=True)
                    rb = smp.tile([64, 512], F32, name="rb")
                    nc.scalar.copy(rb[:], rb_ps[0:64, :])
                    nc.vector.tensor_mul(saT[:, j, s * 512:(s + 1) * 512],
                                         pv_ps[0:64, :], rb[:])

            # ---- output projection (row-parallel partial) ----
            for nt in range(NT):
                for e2 in range(2):
                    pr_ps = psM.tile([128, 512], F32, name="pr_ps", tag="misc")
                    for j in range(HEADS_PER_CORE):
                        nc.tensor.matmul(
                            pr_ps[:, 0:384],
                            _mm_ap(saT[:, j, nt * 128:(nt + 1) * 128]),
                            _mm_ap(wp_sb[:, j, e2 * 384:(e2 + 1) * 384]),
                            start=(j == 0), stop=(j == HEADS_PER_CORE - 1),
                        )
                    ot = otp.tile([128, 384], F32, name="ot")
                    nc.scalar.copy(ot[:], pr_ps[:, 0:384])
                    nc.sync.dma_start(
                        out_d[nt * 128:(nt + 1) * 128, e2 * 384:(e2 + 1) * 384], ot[:])

    nc.compile()
    return nc


def _host_shard(x, W_kqv, b_kqv, W_proj, b_proj):
    """Build the 8 per-core input maps."""
    masks = np.zeros((128, 4, 512), dtype=np.float32)
    yy = np.arange(512)[None, :]
    xx = np.arange(128)[:, None]
    for jj in range(4):
        masks[:, jj, :] = (yy >= xx + jj * 128).astype(np.float32)
    ones64 = np.ones((1, 64), dtype=np.float32)

    in_maps = []
    for c in range(N_CORES):
        b = c // 4
        h0 = (c % 4) * HEADS_PER_CORE
        hs = [h0, h0 + 1, h0 + 2]
        xT = np.ascontiguousarray(x[b].T)                       # [768, 2048]

        wqk = np.empty((128, HEADS_PER_CORE, DC, 128), dtype=np.float32)
        bqk = np.empty((128, HEADS_PER_CORE), dtype=np.float32)
        for j, h in enumerate(hs):
            wj = np.concatenate([W_kqv[h, 64:128], W_kqv[h, 0:64]], axis=0)  # [128, 768]
            # wqk[p, j, c, m] = wj[m, c*128+p]
            wqk[:, j, :, :] = wj.T.reshape(DC, 128, 128).transpose(1, 0, 2)
            bqk[:, j] = np.concatenate([b_kqv[h, 64:128], b_kqv[h, 0:64]])

        wv_all = np.zeros((D, 256), dtype=np.float32)
        for j, h in enumerate(hs):
            wv_all[:, j * 64:(j + 1) * 64] = W_kqv[h, 128:192].T
        wv = np.ascontiguousarray(wv_all.reshape(DC, 128, 256).transpose(1, 0, 2))

        wp = np.empty((64, HEADS_PER_CORE, D), dtype=np.float32)
        for j, h in enumerate(hs):
            wp[:, j, :] = W_proj[:, h * 64:(h + 1) * 64].T

        bvb = np.tile(np.concatenate([b_kqv[h, 128:192] for h in hs])[None, :],
                      (128, 1)).astype(np.float32)

        in_maps.append({
            "xT": xT, "wqk": wqk, "wv": wv, "wp": wp,
            "bqk": bqk, "bvb": bvb, "masks": masks, "ones64": ones64,
        })
    return in_maps


def kernel(x, W_kqv, b_kqv, W_proj, b_proj):
    x = np.asarray(x, dtype=np.float32)
    W_kqv = np.asarray(W_kqv, dtype=np.float32)
    b_kqv = np.asarray(b_kqv, dtype=np.float32)
    W_proj = np.asarray(W_proj, dtype=np.float32)
    b_proj = np.asarray(b_proj, dtype=np.float32)

    if "nc" not in _CACHE:
        _CACHE["nc"] = build()
    nc = _CACHE["nc"]

    in_maps = _host_shard(x, W_kqv, b_kqv, W_proj, b_proj)
    res = run_bass_kernel_spmd(nc, in_maps, list(range(N_CORES)))

    out = np.empty((B, N, D), dtype=np.float32)
    for b in range(B):
        acc = res.results[4 * b]["out"].astype(np.float32)
        for c in range(4 * b + 1, 4 * b + 4):
            acc = acc + res.results[c]["out"]
        out[b] = acc + b_proj[None, :]
    return out


# revision 5
# speedup vs baseline: 2.2280x; 2.2280x over previous
"""Causal self-attention (B=2, N=2048, D=768, H=12, HD=64) on 8 TRN2 NeuronCores.

Sharding: tensor-parallel over (batch, head). Core c handles batch b = c//4 and
heads [3*(c%4), 3*(c%4)+3). Each core computes its 3 heads' attention plus the
matching 192 columns of the output projection (row-parallel W_proj), returning a
partial [2048, 768] output. Host sums the 4 partials per batch element and adds
b_proj.

Per-core kernel layout (all fp32):
  - x arrives transposed (xT [768, 2048]) so the KQV projection produces q/k
    directly in [head_dim, token] orientation (q_T, k_T [64, 2048]).
  - scores are computed pre-transposed, S_T[k, q] = k_T^T-slice @ q_T, so the
    softmax denominator is a matmul reduction: v is padded with a ones column
    and P_T = exp(S_T/8) feeds sa_T[d, q] / denom[q] in one accumulation chain.
  - causal masking is multiplicative on P_T with 4 static [128, 512] masks
    (only diagonal blocks need it; fully-masked blocks are skipped).
  - sa_T [64, 2048] per head is exactly the lhsT the projection needs.
"""

import numpy as np

import concourse.bass as bass
import concourse.mybir as mybir
import concourse.tile as tile
from concourse import bacc
from concourse.bass_utils import run_bass_kernel_spmd

F32 = mybir.dt.float32
AF = mybir.ActivationFunctionType

B, N, D = 2, 2048, 768
H, HD = 12, 64
HEADS_PER_CORE = 3
N_CORES = 8
NT = N // 128          # 16 token tiles of 128
NS = N // 512          # 4 query spans of 512
DC = D // 128          # 6 contraction chunks of 128

# Matmul operand dtype knob: float32 (exact, 4 cycles/row) or float32r
# (1 cycle/row at free-dim >= 256, reduced precision on HW).
MM_DT = mybir.dt.float32r

_CACHE = {}




def build():
    nc = bacc.Bacc("TRN2", target_bir_lowering=False, debug=False)

    xT_d = nc.dram_tensor("xT", [D, N], MM_DT, kind="ExternalInput").ap()
    wqk_d = nc.dram_tensor("wqk", [128, HEADS_PER_CORE, DC, 128], MM_DT, kind="ExternalInput").ap()
    wv_d = nc.dram_tensor("wv", [128, DC, 256], MM_DT, kind="ExternalInput").ap()
    wp_d = nc.dram_tensor("wp", [64, HEADS_PER_CORE, D], MM_DT, kind="ExternalInput").ap()
    bqk_d = nc.dram_tensor("bqk", [128, HEADS_PER_CORE], F32, kind="ExternalInput").ap()
    bvb_d = nc.dram_tensor("bvb", [128, 192], F32, kind="ExternalInput").ap()
    mask_d = nc.dram_tensor("masks", [128, 4, 512], F32, kind="ExternalInput").ap()
    ones_d = nc.dram_tensor("ones64", [1, 64], MM_DT, kind="ExternalInput").ap()
    out_d = nc.dram_tensor("out", [N, D], F32, kind="ExternalOutput").ap()

    with tile.TileContext(nc) as tc, \
         nc.allow_low_precision(reason="fp32r matmul operands; accumulation stays fp32"):
        with tc.tile_pool(name="cn", bufs=1) as cn, \
             tc.tile_pool(name="qk", bufs=2) as qkp, \
             tc.tile_pool(name="pt", bufs=4) as ptp, \
             tc.tile_pool(name="sm", bufs=2) as smp, \
             tc.tile_pool(name="ot", bufs=3) as otp, \
             tc.tile_pool(name="psS", bufs=2, space="PSUM") as psS, \
             tc.tile_pool(name="psP", bufs=4, space="PSUM") as psP, \
             tc.tile_pool(name="psM", bufs=2, space="PSUM") as psM:

            # ---- constant loads ----
            x_sb = cn.tile([128, DC, N], MM_DT, name="x_sb")
            nc.sync.dma_start(x_sb[:], xT_d.rearrange("(c p) n -> p c n", p=128))
            wqk_sb = cn.tile([128, HEADS_PER_CORE, DC, 128], MM_DT, name="wqk_sb")
            nc.sync.dma_start(wqk_sb[:], wqk_d)
            wv_sb = cn.tile([128, DC, 256], MM_DT, name="wv_sb")
            nc.sync.dma_start(wv_sb[:], wv_d)
            wp_sb = cn.tile([64, HEADS_PER_CORE, D], MM_DT, name="wp_sb")
            nc.sync.dma_start(wp_sb[:], wp_d)
            bqk_sb = cn.tile([128, HEADS_PER_CORE], F32, name="bqk_sb")
            nc.sync.dma_start(bqk_sb[:], bqk_d)
            bvb_sb = cn.tile([128, 192], F32, name="bvb_sb")
            nc.sync.dma_start(bvb_sb[:], bvb_d)
            mask_sb = cn.tile([128, 4, 512], F32, name="mask_sb")
            nc.sync.dma_start(mask_sb[:], mask_d)
            ones_sb = cn.tile([1, 64], MM_DT, name="ones_sb")
            nc.sync.dma_start(ones_sb[:], ones_d)

            vf = cn.tile([128, NT, HEADS_PER_CORE, 65], MM_DT, name="vf")
            saT = cn.tile([64, HEADS_PER_CORE, N], MM_DT, name="saT")

            # ---- V projection for all 3 heads fused: v[n, o] o in [0, 192) ----
            for nt in range(NT):
                v_ps = psM.tile([128, 512], F32, name="v_ps", tag="misc")
                for c in range(DC):
                    nc.tensor.matmul(
                        v_ps[:, 0:256],
                        (x_sb[:, c, nt * 128:(nt + 1) * 128]),
                        (wv_sb[:, c, :]),
                        start=(c == 0), stop=(c == DC - 1),
                    )
                nc.vector.tensor_tensor(
                    vf[:, nt, :, 0:64],
                    v_ps[:, 0:192].rearrange("p (h d) -> p h d", h=3),
                    bvb_sb[:].rearrange("p (h d) -> p h d", h=3),
                    op=mybir.AluOpType.add,
                )
                nc.vector.memset(vf[:, nt, :, 64:65].bitcast(F32), 1.0)

            # ---- per-head attention ----
            for j in range(HEADS_PER_CORE):
                qt = qkp.tile([64, N], MM_DT, name="qt")
                kt = qkp.tile([64, N], MM_DT, name="kt")
                for s in range(NS):
                    qk_ps = psM.tile([128, 512], F32, name="qk_ps", tag="misc")
                    for c in range(DC):
                        nc.tensor.matmul(
                            qk_ps[:],
                            (wqk_sb[:, j, c, :]),
                            (x_sb[:, c, s * 512:(s + 1) * 512]),
                            start=(c == 0), stop=(c == DC - 1),
                        )
                    nc.scalar.activation(qt[:, s * 512:(s + 1) * 512], qk_ps[0:64, :],
                                         AF.Identity, bias=bqk_sb[0:64, j:j + 1])
                    nc.scalar.activation(kt[:, s * 512:(s + 1) * 512], qk_ps[64:128, :],
                                         AF.Identity, bias=bqk_sb[64:128, j:j + 1])

                for s in range(NS):
                    nkt = 4 * s + 4
                    pv_ps = psP.tile([65, 512], F32, name="pv_ps", tag="pv")
                    for ktile in range(nkt):
                        sc_ps = psS.tile([128, 512], F32, name="sc_ps", tag="sc")
                        nc.tensor.matmul(
                            sc_ps[:],
                            (kt[:, ktile * 128:(ktile + 1) * 128]),
                            (qt[:, s * 512:(s + 1) * 512]),
                            start=True, stop=True,
                        )
                        pt = ptp.tile([128, 512], MM_DT, name="pt")
                        nc.scalar.activation(pt[:], sc_ps[:], AF.Exp, scale=0.125)
                        if ktile >= 4 * s:
                            nc.vector.tensor_mul(pt[:], pt[:], mask_sb[:, ktile - 4 * s, :])
                        nc.tensor.matmul(
                            pv_ps[:],
                            (vf[:, ktile, j, :]),
                            (pt[:]),
                            start=(ktile == 0), stop=(ktile == nkt - 1),
                        )
                    rc = smp.tile([1, 512], MM_DT, name="rc")
                    nc.vector.reciprocal(rc[:], pv_ps[64:65, :])
                    rb_ps = psM.tile([128, 512], F32, name="rb_ps", tag="misc")
                    nc.tensor.matmul(rb_ps[0:64, :], (ones_sb[:]), (rc[:]),
                                     start=True, stop=True)
                    rb = smp.tile([64, 512], F32, name="rb")
                    nc.scalar.copy(rb[:], rb_ps[0:64, :])
                    nc.vector.tensor_mul(saT[:, j, s * 512:(s + 1) * 512],
                                         pv_ps[0:64, :], rb[:])

            # ---- output projection (row-parallel partial) ----
            for nt in range(NT):
                for e2 in range(2):
                    pr_ps = psM.tile([128, 512], F32, name="pr_ps", tag="misc")
                    for j in range(HEADS_PER_CORE):
                        nc.tensor.matmul(
                            pr_ps[:, 0:384],
                            (saT[:, j, nt * 128:(nt + 1) * 128]),
                            (wp_sb[:, j, e2 * 384:(e2 + 1) * 384]),
                            start=(j == 0), stop=(j == HEADS_PER_CORE - 1),
                        )
                    ot = otp.tile([128, 384], F32, name="ot")
                    nc.scalar.copy(ot[:], pr_ps[:, 0:384])
                    nc.sync.dma_start(
                        out_d[nt * 128:(nt + 1) * 128, e2 * 384:(e2 + 1) * 384], ot[:])

    nc.compile()
    return nc


def _host_shard(x, W_kqv, b_kqv, W_proj, b_proj):
    """Build the 8 per-core input maps."""
    masks = np.zeros((128, 4, 512), dtype=np.float32)
    yy = np.arange(512)[None, :]
    xx = np.arange(128)[:, None]
    for jj in range(4):
        masks[:, jj, :] = (yy >= xx + jj * 128).astype(np.float32)
    ones64 = np.ones((1, 64), dtype=np.float32)

    in_maps = []
    for c in range(N_CORES):
        b = c // 4
        h0 = (c % 4) * HEADS_PER_CORE
        hs = [h0, h0 + 1, h0 + 2]
        xT = np.ascontiguousarray(x[b].T)                       # [768, 2048]

        wqk = np.empty((128, HEADS_PER_CORE, DC, 128), dtype=np.float32)
        bqk = np.empty((128, HEADS_PER_CORE), dtype=np.float32)
        for j, h in enumerate(hs):
            wj = np.concatenate([W_kqv[h, 64:128], W_kqv[h, 0:64]], axis=0)  # [128, 768]
            # wqk[p, j, c, m] = wj[m, c*128+p]
            wqk[:, j, :, :] = wj.T.reshape(DC, 128, 128).transpose(1, 0, 2)
            bqk[:, j] = np.concatenate([b_kqv[h, 64:128], b_kqv[h, 0:64]])

        wv_all = np.zeros((D, 256), dtype=np.float32)
        for j, h in enumerate(hs):
            wv_all[:, j * 64:(j + 1) * 64] = W_kqv[h, 128:192].T
        wv = np.ascontiguousarray(wv_all.reshape(DC, 128, 256).transpose(1, 0, 2))

        wp = np.empty((64, HEADS_PER_CORE, D), dtype=np.float32)
        for j, h in enumerate(hs):
            wp[:, j, :] = W_proj[:, h * 64:(h + 1) * 64].T

        bvb = np.tile(np.concatenate([b_kqv[h, 128:192] for h in hs])[None, :],
                      (128, 1)).astype(np.float32)

        in_maps.append({
            "xT": xT, "wqk": wqk, "wv": wv, "wp": wp,
            "bqk": bqk, "bvb": bvb, "masks": masks, "ones64": ones64,
        })
    return in_maps


def kernel(x, W_kqv, b_kqv, W_proj, b_proj):
    x = np.asarray(x, dtype=np.float32)
    W_kqv = np.asarray(W_kqv, dtype=np.float32)
    b_kqv = np.asarray(b_kqv, dtype=np.float32)
    W_proj = np.asarray(W_proj, dtype=np.float32)
    b_proj = np.asarray(b_proj, dtype=np.float32)

    if "nc" not in _CACHE:
        _CACHE["nc"] = build()
    nc = _CACHE["nc"]

    in_maps = _host_shard(x, W_kqv, b_kqv, W_proj, b_proj)
    res = run_bass_kernel_spmd(nc, in_maps, list(range(N_CORES)))

    out = np.empty((B, N, D), dtype=np.float32)
    for b in range(B):
        acc = res.results[4 * b]["out"].astype(np.float32)
        for c in range(4 * b + 1, 4 * b + 4):
            acc = acc + res.results[c]["out"]
        out[b] = acc + b_proj[None, :]
    return out


# revision 27
# speedup vs baseline: 3.2720x; 1.4686x over previous
"""Causal self-attention (B=2, N=2048, D=768, H=12, HD=64) on 8 TRN2 NeuronCores.

Sharding: tensor-parallel over (batch, head). Core c handles batch b = c//4 and
heads [3*(c%4), 3*(c%4)+3). Each core computes its 3 heads' attention plus the
matching 192 columns of the output projection (row-parallel W_proj), returning a
partial [2048, 768] output. Host sums the 4 partials per batch element and adds
b_proj.

Per-core kernel layout:
  - x arrives transposed (xT [768, 2048]) so the KQV projection produces q/k
    directly in [head_dim, token] orientation; q_T/k_T are written to BOTH
    partition halves of [128, 2048] tiles so score matmuls for even/odd k-tiles
    run concurrently in separate PE row groups (row tiling, K=64 each).
  - scores are computed pre-transposed, S_T[k, q] = k_T-slice.T @ q_T, so the
    softmax denominator is a matmul reduction: v is padded with a ones column
    and P_T = exp(S_T/8) feeds sa_T[d, q] / denom[q] in one accumulation chain.
  - causal structure: fully-masked (k > q) blocks are skipped; diagonal blocks
    are column-trimmed (S_T/exp/mask/PV only touch q >= k_tile_base) and the
    remaining triangle is masked multiplicatively with static [128, 512] masks.
  - sa_T [64, 2048] per head is exactly the lhsT the projection needs; the
    3-head projection accumulation is row-tiled (pair + single, DVE combine).

Matmul-operand tiles are declared float32r (byte-identical to fp32); per-stage
knobs bitcast back to float32 where exact fp32 matmuls are wanted (fp32 runs at
1/4 PE rate, fp32r at full rate for free-dim >= 256 but rounds operands to
reduced precision on HW).
"""

import numpy as np

import concourse.bass as bass
import concourse.mybir as mybir
import concourse.tile as tile
from concourse import bacc
from concourse.alu_op_type import AluOpType
from concourse.bass_utils import run_bass_kernel_spmd

F32 = mybir.dt.float32
F32R = mybir.dt.float32r
AF = mybir.ActivationFunctionType

B, N, D = 2, 2048, 768
H, HD = 12, 64
HEADS_PER_CORE = 3
N_CORES = 8
NT = N // 128          # 16 token tiles of 128
NS = N // 512          # 4 query spans of 512
DC = D // 128          # 6 contraction chunks of 128

# Per-stage matmul dtype: True -> exact fp32 (4 cycles/row), False -> fp32r
# (full rate at free-dim >= 256, reduced precision).
STAGE_F32 = {
    "qk": False,   # kqv q/k projection
    "st": False,   # scores S_T
    "pv": False,   # P^T @ v (+ denominator)
    "v": False,    # v projection
    "pr": False,   # output projection
}

# feature flags; row-tiling only pays off when the stage runs exact fp32
# (4 cyc/row) -- in fp32r mode the extra DVE copies cost more than the PE
# savings. None -> derived from STAGE_F32 at build time.
FLAGS = {"rt_st": None, "trim": True, "rt_pr": None, "gp_rb": True, "gp_mask": False}

_CACHE = {}


def _op(ap, stage):
    return ap.bitcast(F32) if STAGE_F32[stage] else ap


def _dt(stage):
    # dtype for compute-produced tiles: fp32r tiles round values at write time,
    # so only use F32R when the consuming matmul stage runs in fp32r.
    return F32 if STAGE_F32[stage] else F32R


def _ddt(*stages):
    # dtype for DMA-fed tiles/DRAM tensors: F32 unless some consumer is fp32r.
    return F32 if all(STAGE_F32[s] for s in stages) else F32R


def build():
    if FLAGS["rt_st"] is None:
        FLAGS["rt_st"] = STAGE_F32["st"]
    if FLAGS["rt_pr"] is None:
        FLAGS["rt_pr"] = STAGE_F32["pr"]
    nc = bacc.Bacc("TRN2", target_bir_lowering=False, debug=False)

    xT_d = nc.dram_tensor("xT", [D, N], _ddt("qk", "v"), kind="ExternalInput").ap()
    wqk_d = nc.dram_tensor("wqk", [128, HEADS_PER_CORE, DC, 128], _ddt("qk"), kind="ExternalInput").ap()
    wv_d = nc.dram_tensor("wv", [128, DC, 256], _ddt("v"), kind="ExternalInput").ap()
    wp_d = nc.dram_tensor("wp", [64, HEADS_PER_CORE, D], _ddt("pr"), kind="ExternalInput").ap()
    bqk_d = nc.dram_tensor("bqk", [128, HEADS_PER_CORE], F32, kind="ExternalInput").ap()
    bvb_d = nc.dram_tensor("bvb", [128, 192], F32, kind="ExternalInput").ap()
    mask_d = nc.dram_tensor("masks", [128, 4, 512], F32, kind="ExternalInput").ap()
    out_d = nc.dram_tensor("out", [N, D], F32, kind="ExternalOutput").ap()

    vw = 192 if STAGE_F32["v"] else 256   # fp32r wants free-dim >= 256

    with tile.TileContext(nc) as tc, \
         nc.allow_low_precision(reason="fp32r matmul operands; accumulation stays fp32"):
        with tc.tile_pool(name="cn", bufs=1) as cn, \
             tc.tile_pool(name="qk", bufs=2) as qkp, \
             tc.tile_pool(name="pt", bufs=6) as ptp, \
             tc.tile_pool(name="sm", bufs=2) as smp, \
             tc.tile_pool(name="ot", bufs=3) as otp, \
             tc.tile_pool(name="psS", bufs=4, space="PSUM") as psS, \
             tc.tile_pool(name="psP", bufs=2, space="PSUM") as psP, \
             tc.tile_pool(name="psM", bufs=2, space="PSUM") as psM:

            # ---- constant loads ----
            # head-0 qk proj needs x span 0 + its wqk slice first; spread DMAs
            # over both HWDGE queues (SP + ACT) so they land in parallel.
            x_sp = []
            xr = xT_d.rearrange("(c p) n -> p c n", p=128)
            wqk_sb = cn.tile([128, HEADS_PER_CORE, DC, 128], _ddt("qk"), name="wqk_sb")
            bqk_sb = cn.tile([128, HEADS_PER_CORE], F32, name="bqk_sb")
            wv_sb = cn.tile([128, DC, 256], _ddt("v"), name="wv_sb")
            bvb_sb = cn.tile([128, 192], F32, name="bvb_sb")
            wp_sb = cn.tile([64, HEADS_PER_CORE, D], _ddt("pr"), name="wp_sb")
            nc.scalar.dma_start(wqk_sb[:, 0, :, :], wqk_d[:, 0, :, :])
            nc.scalar.dma_start(bqk_sb[:], bqk_d)
            for s in range(NS):
                xs = cn.tile([128, DC, 512], _ddt("qk", "v"), name=f"x_sp{s}")
                # halves of the d-chunks on separate queues so they land in parallel
                xsl = xr[:, :, s * 512:(s + 1) * 512]
                nc.sync.dma_start(xs[:, 0:3, :], xsl[:, 0:3, :])
                nc.scalar.dma_start(xs[:, 3:6, :], xsl[:, 3:6, :])
                x_sp.append(xs)
                if s == 0:
                    nc.scalar.dma_start(wv_sb[:], wv_d)
                    nc.sync.dma_start(bvb_sb[:], bvb_d)
                if s == 1:
                    nc.sync.dma_start(wqk_sb[:, 1:3, :, :], wqk_d[:, 1:3, :, :])
            nc.scalar.dma_start(wp_sb[:], wp_d)
            if not FLAGS["gp_mask"]:
                mask_sb = cn.tile([128, 4, 512], F32, name="mask_sb")
                nc.sync.dma_start(mask_sb[:], mask_d)

            vf = cn.tile([128, NT, HEADS_PER_CORE, 65], _dt("pv"), name="vf")
            saT = cn.tile([64, HEADS_PER_CORE, N], _dt("pr"), name="saT")

            def emit_qk(j):
                # q_T/k_T (duplicated into both partition halves when row tiling)
                qh = 128 if FLAGS["rt_st"] else 64
                qt = qkp.tile([qh, N], _dt("st"), name="qt")
                kt = qkp.tile([qh, N], _dt("st"), name="kt")
                for s in range(NS):
                    qk_ps = psM.tile([128, 512], F32, name="qk_ps", tag="misc")
                    for c in range(DC):
                        nc.tensor.matmul(
                            qk_ps[:],
                            _op(wqk_sb[:, j, c, :], "qk"),
                            _op(x_sp[s][:, c, :], "qk"),
                            start=(c == 0), stop=(c == DC - 1),
                        )
                    sl = slice(s * 512, (s + 1) * 512)
                    nc.vector.tensor_scalar_add(qt[0:64, sl], qk_ps[0:64, :],
                                                bqk_sb[0:64, j:j + 1])
                    nc.vector.tensor_scalar_add(kt[0:64, sl], qk_ps[64:128, :],
                                                bqk_sb[64:128, j:j + 1])
                    if FLAGS["rt_st"]:
                        nc.vector.tensor_scalar_add(qt[64:128, sl], qk_ps[0:64, :],
                                                    bqk_sb[0:64, j:j + 1])
                        nc.vector.tensor_scalar_add(kt[64:128, sl], qk_ps[64:128, :],
                                                    bqk_sb[64:128, j:j + 1])
                return qt, kt

            def emit_v(nt_range):
                # V projection for all 3 heads fused: v[n, o], o in [0, 192)
                for nt in nt_range:
                    v_ps = psM.tile([128, 512], F32, name="v_ps", tag="misc")
                    for c in range(DC):
                        nc.tensor.matmul(
                            v_ps[:, 0:vw],
                            _op(x_sp[nt // 4][:, c, (nt % 4) * 128:(nt % 4 + 1) * 128], "v"),
                            _op(wv_sb[:, c, 0:vw], "v"),
                            start=(c == 0), stop=(c == DC - 1),
                        )
                    nc.vector.tensor_tensor(
                        vf[:, nt, :, 0:64],
                        v_ps[:, 0:192].rearrange("p (h d) -> p h d", h=3),
                        bvb_sb[:].rearrange("p (h d) -> p h d", h=3),
                        op=mybir.AluOpType.add,
                    )
                    nc.vector.memset(vf[:, nt, :, 64:65].bitcast(F32), 1.0)

            def emit_attn(j, qt, kt, s_range):
                for s in s_range:
                    nkt = 4 * s + 4
                    pv_ps = psP.tile([65, 512], F32, name="pv_ps", tag="pv")
                    for kt0 in range(0, nkt, 2):
                        pair = (kt0, kt0 + 1)
                        offs, pts = [], []
                        # S_T for the pair, adjacent on PE, in separate row groups
                        for idx, ktile in enumerate(pair):
                            # diagonal blocks only need columns q >= ktile*128
                            off = max(0, (ktile - 4 * s)) * 128 if FLAGS["trim"] else 0
                            if not STAGE_F32["st"]:
                                off = min(off, 256)
                            offs.append(off)
                            half = idx if FLAGS["rt_st"] else 0
                            lo, hi = (0, 64) if half == 0 else (64, 128)
                            sc_ps = psS.tile([128, 512], F32, name="sc_ps", tag="sc")
                            nc.tensor.matmul(
                                sc_ps[:, off:512],
                                _op(kt[lo:hi, ktile * 128:(ktile + 1) * 128], "st"),
                                _op(qt[lo:hi, s * 512 + off:(s + 1) * 512], "st"),
                                start=True, stop=True,
                                tile_position=(lo, 0),
                            )
                            pt = ptp.tile([128, 512], _dt("pv"), name="pt")
                            nc.scalar.activation(pt[:, off:512], sc_ps[:, off:512],
                                                 AF.Exp, scale=0.125)
                            if ktile >= 4 * s:
                                jj = ktile - 4 * s
                                if FLAGS["gp_mask"]:
                                    nc.gpsimd.affine_select(
                                        out=pt[:, off:512],
                                        in_=pt[:, off:512],
                                        compare_op=AluOpType.is_ge, fill=0.0,
                                        base=off - jj * 128,
                                        pattern=[[1, 512 - off]],
                                        channel_multiplier=-1,
                                    )
                                else:
                                    nc.vector.tensor_mul(pt[:, off:512], pt[:, off:512],
                                                         mask_sb[:, jj, off:512])
                            pts.append(pt)
                        for idx, ktile in enumerate(pair):
                            off = offs[idx]
                            nc.tensor.matmul(
                                pv_ps[:, off:512],
                                _op(vf[:, ktile, j, :], "pv"),
                                _op(pts[idx][:, off:512], "pv"),
                                start=(ktile == 0), stop=(ktile == nkt - 1),
                            )
                    rc = smp.tile([1, 512], F32, name="rc")
                    nc.vector.reciprocal(rc[:], pv_ps[64:65, :])
                    rb = smp.tile([64, 512], F32, name="rb")
                    if FLAGS["gp_rb"]:
                        nc.gpsimd.partition_broadcast(rb[:], rc[:])
                    else:
                        ones_t = smp.tile([1, 64], F32, name="ones_t")
                        nc.vector.memset(ones_t[:], 1.0)
                        rb_ps = psM.tile([128, 512], F32, name="rb_ps", tag="misc")
                        nc.tensor.matmul(rb_ps[0:64, :], ones_t[:], rc[:],
                                         start=True, stop=True)
                        nc.vector.tensor_copy(rb[:], rb_ps[0:64, :])
                    nc.vector.tensor_mul(saT[:, j, s * 512:(s + 1) * 512],
                                         pv_ps[0:64, :], rb[:])

            # head 0 runs span-by-span so only x span s gates span s work
            qt0, kt0 = emit_qk(0)
            for s in range(NS):
                emit_v(range(4 * s, 4 * s + 4))
                emit_attn(0, qt0, kt0, [s])
            for j in range(1, HEADS_PER_CORE):
                qtj, ktj = emit_qk(j)
                emit_attn(j, qtj, ktj, range(NS))

            # ---- output projection (row-parallel partial), row-tiled pair + single ----
            # wp halves: head 0 at partitions 0:64, head 1 at 64:128; head 2 at 0:64 (second slot)
            if FLAGS["rt_pr"]:
                wp2 = cn.tile([128, 2, D], _dt("pr"), name="wp2")
                nc.vector.tensor_copy(_op(wp2[0:64, 0, :], "pr"), _op(wp_sb[:, 0, :], "pr"))
                nc.vector.tensor_copy(_op(wp2[64:128, 0, :], "pr"), _op(wp_sb[:, 1, :], "pr"))
                nc.vector.tensor_copy(_op(wp2[0:64, 1, :], "pr"), _op(wp_sb[:, 2, :], "pr"))
                sa2 = cn.tile([128, 2, N], _dt("pr"), name="sa2")
                nc.vector.tensor_copy(_op(sa2[0:64, 0, :], "pr"), _op(saT[:, 0, :], "pr"))
                nc.vector.tensor_copy(_op(sa2[64:128, 0, :], "pr"), _op(saT[:, 1, :], "pr"))
                nc.vector.tensor_copy(_op(sa2[0:64, 1, :], "pr"), _op(saT[:, 2, :], "pr"))

            for nt in range(NT):
                ntl = slice(nt * 128, (nt + 1) * 128)
                for e2 in range(2):
                    esl = slice(e2 * 384, (e2 + 1) * 384)
                    if FLAGS["rt_pr"]:
                        pA = psM.tile([128, 512], F32, name="pA", tag="misc")
                        pB = psM.tile([128, 512], F32, name="pB", tag="misc")
                        nc.tensor.matmul(pA[:, 0:384], _op(sa2[0:64, 0, ntl], "pr"),
                                         _op(wp2[0:64, 0, esl], "pr"),
                                         start=True, stop=False, tile_position=(0, 0))
                        nc.tensor.matmul(pB[:, 0:384], _op(sa2[64:128, 0, ntl], "pr"),
                                         _op(wp2[64:128, 0, esl], "pr"),
                                         start=True, stop=True, tile_position=(64, 0))
                        nc.tensor.matmul(pA[:, 0:384], _op(sa2[0:64, 1, ntl], "pr"),
                                         _op(wp2[0:64, 1, esl], "pr"),
                                         start=False, stop=True, tile_position=(0, 0))
                        tmp = otp.tile([128, 384], F32, name="tmp")
                        nc.vector.tensor_copy(tmp[:], pB[:, 0:384])
                        ot = otp.tile([128, 384], F32, name="ot")
                        nc.vector.tensor_tensor(ot[:], pA[:, 0:384], tmp[:],
                                                op=mybir.AluOpType.add)
                    else:
                        pA = psM.tile([128, 512], F32, name="pA", tag="misc")
                        for j in range(HEADS_PER_CORE):
                            nc.tensor.matmul(
                                pA[:, 0:384],
                                _op(saT[:, j, ntl], "pr"),
                                _op(wp_sb[:, j, esl], "pr"),
                                start=(j == 0), stop=(j == HEADS_PER_CORE - 1),
                            )
                        ot = otp.tile([128, 384], F32, name="ot")
                        nc.vector.tensor_copy(ot[:], pA[:, 0:384])
                    nc.sync.dma_start(out_d[ntl, esl], ot[:])

    nc.compile()
    return nc


def _host_shard(x, W_kqv, b_kqv, W_proj, b_proj):
    """Build the 8 per-core input maps."""
    masks = np.zeros((128, 4, 512), dtype=np.float32)
    yy = np.arange(512)[None, :]
    xx = np.arange(128)[:, None]
    for jj in range(4):
        masks[:, jj, :] = (yy >= xx + jj * 128).astype(np.float32)

    in_maps = []
    for c in range(N_CORES):
        b = c // 4
        h0 = (c % 4) * HEADS_PER_CORE
        hs = [h0, h0 + 1, h0 + 2]
        xT = np.ascontiguousarray(x[b].T)                       # [768, 2048]

        wqk = np.empty((128, HEADS_PER_CORE, DC, 128), dtype=np.float32)
        bqk = np.empty((128, HEADS_PER_CORE), dtype=np.float32)
        for j, h in enumerate(hs):
            wj = np.concatenate([W_kqv[h, 64:128], W_kqv[h, 0:64]], axis=0)  # [128, 768]
            # wqk[p, j, c, m] = wj[m, c*128+p]
            wqk[:, j, :, :] = wj.T.reshape(DC, 128, 128).transpose(1, 0, 2)
            bqk[:, j] = np.concatenate([b_kqv[h, 64:128], b_kqv[h, 0:64]])

        wv_all = np.zeros((D, 256), dtype=np.float32)
        for j, h in enumerate(hs):
            wv_all[:, j * 64:(j + 1) * 64] = W_kqv[h, 128:192].T
        wv = np.ascontiguousarray(wv_all.reshape(DC, 128, 256).transpose(1, 0, 2))

        wp = np.empty((64, HEADS_PER_CORE, D), dtype=np.float32)
        for j, h in enumerate(hs):
            wp[:, j, :] = W_proj[:, h * 64:(h + 1) * 64].T

        bvb = np.tile(np.concatenate([b_kqv[h, 128:192] for h in hs])[None, :],
                      (128, 1)).astype(np.float32)

        in_maps.append({
            "xT": xT, "wqk": wqk, "wv": wv, "wp": wp,
            "bqk": bqk, "bvb": bvb, "masks": masks,
        })
    return in_maps


def kernel(x, W_kqv, b_kqv, W_proj, b_proj):
    x = np.asarray(x, dtype=np.float32)
    W_kqv = np.asarray(W_kqv, dtype=np.float32)
    b_kqv = np.asarray(b_kqv, dtype=np.float32)
    W_proj = np.asarray(W_proj, dtype=np.float32)
    b_proj = np.asarray(b_proj, dtype=np.float32)

    if "nc" not in _CACHE:
        _CACHE["nc"] = build()
    nc = _CACHE["nc"]

    in_maps = _host_shard(x, W_kqv, b_kqv, W_proj, b_proj)
    res = run_bass_kernel_spmd(nc, in_maps, list(range(N_CORES)))

    out = np.empty((B, N, D), dtype=np.float32)
    for b in range(B):
        acc = res.results[4 * b]["out"].astype(np.float32)
        for c in range(4 * b + 1, 4 * b + 4):
            acc = acc + res.results[c]["out"]
        out[b] = acc + b_proj[None, :]
    return out


# revision 33
# speedup vs baseline: 3.3500x; 1.0238x over previous
"""Causal self-attention (B=2, N=2048, D=768, H=12, HD=64) on 8 TRN2 NeuronCores.

Sharding: tensor-parallel over (batch, head). Core c handles batch b = c//4 and
heads [3*(c%4), 3*(c%4)+3). Each core computes its 3 heads' attention plus the
matching 192 columns of the output projection (row-parallel W_proj), returning a
partial [2048, 768] output. Host sums the 4 partials per batch element and adds
b_proj.

Per-core kernel layout:
  - x arrives transposed (xT [768, 2048]) so the KQV projection produces q/k
    directly in [head_dim, token] orientation; q_T/k_T are written to BOTH
    partition halves of [128, 2048] tiles so score matmuls for even/odd k-tiles
    run concurrently in separate PE row groups (row tiling, K=64 each).
  - scores are computed pre-transposed, S_T[k, q] = k_T-slice.T @ q_T, so the
    softmax denominator is a matmul reduction: v is padded with a ones column
    and P_T = exp(S_T/8) feeds sa_T[d, q] / denom[q] in one accumulation chain.
  - causal structure: fully-masked (k > q) blocks are skipped; diagonal blocks
    are column-trimmed (S_T/exp/mask/PV only touch q >= k_tile_base) and the
    remaining triangle is masked multiplicatively with static [128, 512] masks.
  - sa_T [64, 2048] per head is exactly the lhsT the projection needs; the
    3-head projection accumulation is row-tiled (pair + single, DVE combine).

Matmul-operand tiles are declared float32r (byte-identical to fp32); per-stage
knobs bitcast back to float32 where exact fp32 matmuls are wanted (fp32 runs at
1/4 PE rate, fp32r at full rate for free-dim >= 256 but rounds operands to
reduced precision on HW).
"""

import numpy as np

import concourse.bass as bass
import concourse.mybir as mybir
import concourse.tile as tile
from concourse import bacc
from concourse.alu_op_type import AluOpType
from concourse.bass_utils import run_bass_kernel_spmd

F32 = mybir.dt.float32
F32R = mybir.dt.float32r
AF = mybir.ActivationFunctionType

B, N, D = 2, 2048, 768
H, HD = 12, 64
HEADS_PER_CORE = 3
N_CORES = 8
NT = N // 128          # 16 token tiles of 128
NS = N // 512          # 4 query spans of 512
DC = D // 128          # 6 contraction chunks of 128

# Per-stage matmul dtype: True -> exact fp32 (4 cycles/row), False -> fp32r
# (full rate at free-dim >= 256, reduced precision).
STAGE_F32 = {
    "qk": False,   # kqv q/k projection
    "st": False,   # scores S_T
    "pv": False,   # P^T @ v (+ denominator)
    "v": False,    # v projection
    "pr": False,   # output projection
}

# feature flags; row-tiling only pays off when the stage runs exact fp32
# (4 cyc/row) -- in fp32r mode the extra DVE copies cost more than the PE
# savings. None -> derived from STAGE_F32 at build time.
FLAGS = {"rt_st": None, "trim": True, "rt_pr": None, "gp_rb": True, "gp_mask": False}

_CACHE = {}


def _op(ap, stage):
    return ap.bitcast(F32) if STAGE_F32[stage] else ap


def _dt(stage):
    # dtype for compute-produced tiles: fp32r tiles round values at write time,
    # so only use F32R when the consuming matmul stage runs in fp32r.
    return F32 if STAGE_F32[stage] else F32R


def _ddt(*stages):
    # dtype for DMA-fed tiles/DRAM tensors: F32 unless some consumer is fp32r.
    return F32 if all(STAGE_F32[s] for s in stages) else F32R


def build():
    if FLAGS["rt_st"] is None:
        FLAGS["rt_st"] = STAGE_F32["st"]
    if FLAGS["rt_pr"] is None:
        FLAGS["rt_pr"] = STAGE_F32["pr"]
    nc = bacc.Bacc("TRN2", target_bir_lowering=False, debug=False)

    xT_d = nc.dram_tensor("xT", [D, N], _ddt("qk", "v"), kind="ExternalInput").ap()
    wqk_d = nc.dram_tensor("wqk", [128, HEADS_PER_CORE, DC, 128], _ddt("qk"), kind="ExternalInput").ap()
    wv_d = nc.dram_tensor("wv", [128, DC, 256], _ddt("v"), kind="ExternalInput").ap()
    wp_d = nc.dram_tensor("wp", [64, HEADS_PER_CORE, D], _ddt("pr"), kind="ExternalInput").ap()
    bqk_d = nc.dram_tensor("bqk", [128, HEADS_PER_CORE], F32, kind="ExternalInput").ap()
    bvb_d = nc.dram_tensor("bvb", [128, 192], F32, kind="ExternalInput").ap()
    mask_d = nc.dram_tensor("masks", [128, 4, 512], F32, kind="ExternalInput").ap()
    out_d = nc.dram_tensor("out", [N, D], F32, kind="ExternalOutput").ap()

    vw = 192 if STAGE_F32["v"] else 256   # fp32r wants free-dim >= 256

    with tile.TileContext(nc) as tc, \
         nc.allow_low_precision(reason="fp32r matmul operands; accumulation stays fp32"):
        with tc.tile_pool(name="cn", bufs=1) as cn, \
             tc.tile_pool(name="qk", bufs=2) as qkp, \
             tc.tile_pool(name="pt", bufs=6) as ptp, \
             tc.tile_pool(name="sm", bufs=2) as smp, \
             tc.tile_pool(name="ot", bufs=3) as otp, \
             tc.tile_pool(name="psS", bufs=4, space="PSUM") as psS, \
             tc.tile_pool(name="psP", bufs=2, space="PSUM") as psP, \
             tc.tile_pool(name="psM", bufs=2, space="PSUM") as psM:

            # ---- constant loads ----
            # head-0 qk proj needs x span 0 + its wqk slice first; spread DMAs
            # over both HWDGE queues (SP + ACT) so they land in parallel.
            x_sp = []
            xr = xT_d.rearrange("(c p) n -> p c n", p=128)
            wqk_sb = cn.tile([128, HEADS_PER_CORE, DC, 128], _ddt("qk"), name="wqk_sb")
            bqk_sb = cn.tile([128, HEADS_PER_CORE], F32, name="bqk_sb")
            wv_sb = cn.tile([128, DC, 256], _ddt("v"), name="wv_sb")
            bvb_sb = cn.tile([128, 192], F32, name="bvb_sb")
            wp_sb = cn.tile([64, HEADS_PER_CORE, D], _ddt("pr"), name="wp_sb")
            nc.scalar.dma_start(wqk_sb[:, 0, :, :], wqk_d[:, 0, :, :])
            nc.scalar.dma_start(bqk_sb[:], bqk_d)
            for s in range(NS):
                xs = cn.tile([128, DC, 512], _ddt("qk", "v"), name=f"x_sp{s}")
                # halves of the d-chunks on separate queues so they land in parallel
                xsl = xr[:, :, s * 512:(s + 1) * 512]
                nc.sync.dma_start(xs[:, 0:3, :], xsl[:, 0:3, :])
                nc.scalar.dma_start(xs[:, 3:6, :], xsl[:, 3:6, :])
                x_sp.append(xs)
                if s == 0:
                    nc.scalar.dma_start(wv_sb[:], wv_d)
                    nc.sync.dma_start(bvb_sb[:], bvb_d)
                if s == 1:
                    nc.sync.dma_start(wqk_sb[:, 1:3, :, :], wqk_d[:, 1:3, :, :])
            nc.scalar.dma_start(wp_sb[:], wp_d)
            if not FLAGS["gp_mask"]:
                mask_sb = cn.tile([128, 4, 512], F32, name="mask_sb")
                nc.sync.dma_start(mask_sb[:], mask_d)

            vf = cn.tile([128, NT, HEADS_PER_CORE, 65], _dt("pv"), name="vf")
            saT = cn.tile([64, HEADS_PER_CORE, N], _dt("pr"), name="saT")

            def emit_qk(j):
                # q_T/k_T (duplicated into both partition halves when row tiling)
                qh = 128 if FLAGS["rt_st"] else 64
                qt = qkp.tile([qh, N], _dt("st"), name="qt")
                kt = qkp.tile([qh, N], _dt("st"), name="kt")
                for s in range(NS):
                    qk_ps = psM.tile([128, 512], F32, name="qk_ps", tag="misc")
                    for c in range(DC):
                        nc.tensor.matmul(
                            qk_ps[:],
                            _op(wqk_sb[:, j, c, :], "qk"),
                            _op(x_sp[s][:, c, :], "qk"),
                            start=(c == 0), stop=(c == DC - 1),
                        )
                    sl = slice(s * 512, (s + 1) * 512)
                    nc.vector.tensor_scalar_add(qt[0:64, sl], qk_ps[0:64, :],
                                                bqk_sb[0:64, j:j + 1])
                    nc.vector.tensor_scalar_add(kt[0:64, sl], qk_ps[64:128, :],
                                                bqk_sb[64:128, j:j + 1])
                    if FLAGS["rt_st"]:
                        nc.vector.tensor_scalar_add(qt[64:128, sl], qk_ps[0:64, :],
                                                    bqk_sb[0:64, j:j + 1])
                        nc.vector.tensor_scalar_add(kt[64:128, sl], qk_ps[64:128, :],
                                                    bqk_sb[64:128, j:j + 1])
                return qt, kt

            def emit_v(nt_range):
                # V projection for all 3 heads fused: v[n, o], o in [0, 192)
                for nt in nt_range:
                    v_ps = psM.tile([128, 512], F32, name="v_ps", tag="misc")
                    for c in range(DC):
                        nc.tensor.matmul(
                            v_ps[:, 0:vw],
                            _op(x_sp[nt // 4][:, c, (nt % 4) * 128:(nt % 4 + 1) * 128], "v"),
                            _op(wv_sb[:, c, 0:vw], "v"),
                            start=(c == 0), stop=(c == DC - 1),
                        )
                    nc.vector.tensor_tensor(
                        vf[:, nt, :, 0:64],
                        v_ps[:, 0:192].rearrange("p (h d) -> p h d", h=3),
                        bvb_sb[:].rearrange("p (h d) -> p h d", h=3),
                        op=mybir.AluOpType.add,
                    )
                    nc.vector.memset(vf[:, nt, :, 64:65].bitcast(F32), 1.0)

            def emit_attn(j, qt, kt, s_range):
                for s in s_range:
                    nkt = 4 * s + 4
                    pv_ps = psP.tile([65, 512], F32, name="pv_ps", tag="pv")
                    for kt0 in range(0, nkt, 2):
                        pair = (kt0, kt0 + 1)
                        offs, pts = [], []
                        # S_T for the pair, adjacent on PE, in separate row groups
                        for idx, ktile in enumerate(pair):
                            # diagonal blocks only need columns q >= ktile*128
                            off = max(0, (ktile - 4 * s)) * 128 if FLAGS["trim"] else 0
                            if not STAGE_F32["st"]:
                                off = min(off, 256)
                            offs.append(off)
                            half = idx if FLAGS["rt_st"] else 0
                            lo, hi = (0, 64) if half == 0 else (64, 128)
                            sc_ps = psS.tile([128, 512], F32, name="sc_ps", tag="sc")
                            nc.tensor.matmul(
                                sc_ps[:, off:512],
                                _op(kt[lo:hi, ktile * 128:(ktile + 1) * 128], "st"),
                                _op(qt[lo:hi, s * 512 + off:(s + 1) * 512], "st"),
                                start=True, stop=True,
                                tile_position=(lo, 0),
                            )
                            pt = ptp.tile([128, 512], _dt("pv"), name="pt")
                            nc.scalar.activation(pt[:, off:512], sc_ps[:, off:512],
                                                 AF.Exp, scale=0.125)
                            if ktile >= 4 * s:
                                jj = ktile - 4 * s
                                if FLAGS["gp_mask"]:
                                    nc.gpsimd.affine_select(
                                        out=pt[:, off:512],
                                        in_=pt[:, off:512],
                                        compare_op=AluOpType.is_ge, fill=0.0,
                                        base=off - jj * 128,
                                        pattern=[[1, 512 - off]],
                                        channel_multiplier=-1,
                                    )
                                else:
                                    nc.vector.tensor_mul(pt[:, off:512], pt[:, off:512],
                                                         mask_sb[:, jj, off:512])
                            pts.append(pt)
                        for idx, ktile in enumerate(pair):
                            off = offs[idx]
                            nc.tensor.matmul(
                                pv_ps[:, off:512],
                                _op(vf[:, ktile, j, :], "pv"),
                                _op(pts[idx][:, off:512], "pv"),
                                start=(ktile == 0), stop=(ktile == nkt - 1),
                            )
                    rc = smp.tile([1, 512], F32, name="rc")
                    nc.vector.reciprocal(rc[:], pv_ps[64:65, :])
                    rb = smp.tile([64, 512], F32, name="rb")
                    if FLAGS["gp_rb"]:
                        nc.gpsimd.partition_broadcast(rb[:], rc[:])
                    else:
                        ones_t = smp.tile([1, 64], F32, name="ones_t")
                        nc.vector.memset(ones_t[:], 1.0)
                        rb_ps = psM.tile([128, 512], F32, name="rb_ps", tag="misc")
                        nc.tensor.matmul(rb_ps[0:64, :], ones_t[:], rc[:],
                                         start=True, stop=True)
                        nc.vector.tensor_copy(rb[:], rb_ps[0:64, :])
                    nc.vector.tensor_mul(saT[:, j, s * 512:(s + 1) * 512],
                                         pv_ps[0:64, :], rb[:])

            def emit_proj(nt_range):
                for nt in nt_range:
                    ntl = slice(nt * 128, (nt + 1) * 128)
                    for e2 in range(2):
                        esl = slice(e2 * 384, (e2 + 1) * 384)
                        pA = psM.tile([128, 512], F32, name="pA", tag="misc")
                        for j in range(HEADS_PER_CORE):
                            nc.tensor.matmul(
                                pA[:, 0:384],
                                _op(saT[:, j, ntl], "pr"),
                                _op(wp_sb[:, j, esl], "pr"),
                                start=(j == 0), stop=(j == HEADS_PER_CORE - 1),
                            )
                        ot = otp.tile([128, 384], F32, name="ot")
                        nc.vector.tensor_copy(ot[:], pA[:, 0:384])
                        nc.sync.dma_start(out_d[ntl, esl], ot[:])

            if not FLAGS["rt_pr"]:
                qt0, kt0 = emit_qk(0)
                for s in range(NS):
                    emit_v(range(4 * s, 4 * s + 4))
                    emit_attn(0, qt0, kt0, [s])
                qt1, kt1 = emit_qk(1)
                emit_attn(1, qt1, kt1, range(NS))
                qt2, kt2 = emit_qk(2)
                for s in range(NS):
                    emit_attn(2, qt2, kt2, [s])
                    emit_proj(range(4 * s, 4 * s + 4))
                emitted_proj = True
            else:
                qt0, kt0 = emit_qk(0)
                for s in range(NS):
                    emit_v(range(4 * s, 4 * s + 4))
                    emit_attn(0, qt0, kt0, [s])
                for j in range(1, HEADS_PER_CORE):
                    qtj, ktj = emit_qk(j)
                    emit_attn(j, qtj, ktj, range(NS))
                emitted_proj = False

            # ---- output projection (row-tiled variant; spans-outer path did it) ----
            # wp halves: head 0 at partitions 0:64, head 1 at 64:128; head 2 at 0:64 (second slot)
            if FLAGS["rt_pr"]:
                wp2 = cn.tile([128, 2, D], _dt("pr"), name="wp2")
                nc.vector.tensor_copy(_op(wp2[0:64, 0, :], "pr"), _op(wp_sb[:, 0, :], "pr"))
                nc.vector.tensor_copy(_op(wp2[64:128, 0, :], "pr"), _op(wp_sb[:, 1, :], "pr"))
                nc.vector.tensor_copy(_op(wp2[0:64, 1, :], "pr"), _op(wp_sb[:, 2, :], "pr"))
                sa2 = cn.tile([128, 2, N], _dt("pr"), name="sa2")
                nc.vector.tensor_copy(_op(sa2[0:64, 0, :], "pr"), _op(saT[:, 0, :], "pr"))
                nc.vector.tensor_copy(_op(sa2[64:128, 0, :], "pr"), _op(saT[:, 1, :], "pr"))
                nc.vector.tensor_copy(_op(sa2[0:64, 1, :], "pr"), _op(saT[:, 2, :], "pr"))

            for nt in (range(NT) if not emitted_proj else ()):
                ntl = slice(nt * 128, (nt + 1) * 128)
                for e2 in range(2):
                    esl = slice(e2 * 384, (e2 + 1) * 384)
                    if FLAGS["rt_pr"]:
                        pA = psM.tile([128, 512], F32, name="pA", tag="misc")
                        pB = psM.tile([128, 512], F32, name="pB", tag="misc")
                        nc.tensor.matmul(pA[:, 0:384], _op(sa2[0:64, 0, ntl], "pr"),
                                         _op(wp2[0:64, 0, esl], "pr"),
                                         start=True, stop=False, tile_position=(0, 0))
                        nc.tensor.matmul(pB[:, 0:384], _op(sa2[64:128, 0, ntl], "pr"),
                                         _op(wp2[64:128, 0, esl], "pr"),
                                         start=True, stop=True, tile_position=(64, 0))
                        nc.tensor.matmul(pA[:, 0:384], _op(sa2[0:64, 1, ntl], "pr"),
                                         _op(wp2[0:64, 1, esl], "pr"),
                                         start=False, stop=True, tile_position=(0, 0))
                        tmp = otp.tile([128, 384], F32, name="tmp")
                        nc.vector.tensor_copy(tmp[:], pB[:, 0:384])
                        ot = otp.tile([128, 384], F32, name="ot")
                        nc.vector.tensor_tensor(ot[:], pA[:, 0:384], tmp[:],
                                                op=mybir.AluOpType.add)
                    else:
                        pA = psM.tile([128, 512], F32, name="pA", tag="misc")
                        for j in range(HEADS_PER_CORE):
                            nc.tensor.matmul(
                                pA[:, 0:384],
                                _op(saT[:, j, ntl], "pr"),
                                _op(wp_sb[:, j, esl], "pr"),
                                start=(j == 0), stop=(j == HEADS_PER_CORE - 1),
                            )
                        ot = otp.tile([128, 384], F32, name="ot")
                        nc.vector.tensor_copy(ot[:], pA[:, 0:384])
                    nc.sync.dma_start(out_d[ntl, esl], ot[:])

    nc.compile()
    return nc


def _host_shard(x, W_kqv, b_kqv, W_proj, b_proj):
    """Build the 8 per-core input maps."""
    masks = np.zeros((128, 4, 512), dtype=np.float32)
    yy = np.arange(512)[None, :]
    xx = np.arange(128)[:, None]
    for jj in range(4):
        masks[:, jj, :] = (yy >= xx + jj * 128).astype(np.float32)

    in_maps = []
    for c in range(N_CORES):
        b = c // 4
        h0 = (c % 4) * HEADS_PER_CORE
        hs = [h0, h0 + 1, h0 + 2]
        xT = np.ascontiguousarray(x[b].T)                       # [768, 2048]

        wqk = np.empty((128, HEADS_PER_CORE, DC, 128), dtype=np.float32)
        bqk = np.empty((128, HEADS_PER_CORE), dtype=np.float32)
        for j, h in enumerate(hs):
            wj = np.concatenate([W_kqv[h, 64:128], W_kqv[h, 0:64]], axis=0)  # [128, 768]
            # wqk[p, j, c, m] = wj[m, c*128+p]
            wqk[:, j, :, :] = wj.T.reshape(DC, 128, 128).transpose(1, 0, 2)
            bqk[:, j] = np.concatenate([b_kqv[h, 64:128], b_kqv[h, 0:64]])

        wv_all = np.zeros((D, 256), dtype=np.float32)
        for j, h in enumerate(hs):
            wv_all[:, j * 64:(j + 1) * 64] = W_kqv[h, 128:192].T
        wv = np.ascontiguousarray(wv_all.reshape(DC, 128, 256).transpose(1, 0, 2))

        wp = np.empty((64, HEADS_PER_CORE, D), dtype=np.float32)
        for j, h in enumerate(hs):
            wp[:, j, :] = W_proj[:, h * 64:(h + 1) * 64].T

        bvb = np.tile(np.concatenate([b_kqv[h, 128:192] for h in hs])[None, :],
                      (128, 1)).astype(np.float32)

        in_maps.append({
            "xT": xT, "wqk": wqk, "wv": wv, "wp": wp,
            "bqk": bqk, "bvb": bvb, "masks": masks,
        })
    return in_maps


def kernel(x, W_kqv, b_kqv, W_proj, b_proj):
    x = np.asarray(x, dtype=np.float32)
    W_kqv = np.asarray(W_kqv, dtype=np.float32)
    b_kqv = np.asarray(b_kqv, dtype=np.float32)
    W_proj = np.asarray(W_proj, dtype=np.float32)
    b_proj = np.asarray(b_proj, dtype=np.float32)

    if "nc" not in _CACHE:
        _CACHE["nc"] = build()
    nc = _CACHE["nc"]

    in_maps = _host_shard(x, W_kqv, b_kqv, W_proj, b_proj)
    res = run_bass_kernel_spmd(nc, in_maps, list(range(N_CORES)))

    out = np.empty((B, N, D), dtype=np.float32)
    for b in range(B):
        acc = res.results[4 * b]["out"].astype(np.float32)
        for c in range(4 * b + 1, 4 * b + 4):
            acc = acc + res.results[c]["out"]
        out[b] = acc + b_proj[None, :]
    return out


# revision 37
# speedup vs baseline: 3.5072x; 1.0469x over previous
"""Causal self-attention (B=2, N=2048, D=768, H=12, HD=64) on 8 TRN2 NeuronCores.

Sharding: tensor-parallel over (batch, head). Core c handles batch b = c//4 and
heads [3*(c%4), 3*(c%4)+3). Each core computes its 3 heads' attention plus the
matching 192 columns of the output projection (row-parallel W_proj), returning a
partial [2048, 768] output. Host sums the 4 partials per batch element and adds
b_proj.

Per-core kernel layout:
  - x arrives transposed (xT [768, 2048]) so the KQV projection produces q/k
    directly in [head_dim, token] orientation; q_T/k_T are written to BOTH
    partition halves of [128, 2048] tiles so score matmuls for even/odd k-tiles
    run concurrently in separate PE row groups (row tiling, K=64 each).
  - scores are computed pre-transposed, S_T[k, q] = k_T-slice.T @ q_T, so the
    softmax denominator is a matmul reduction: v is padded with a ones column
    and P_T = exp(S_T/8) feeds sa_T[d, q] / denom[q] in one accumulation chain.
  - causal structure: fully-masked (k > q) blocks are skipped; diagonal blocks
    are column-trimmed (S_T/exp/mask/PV only touch q >= k_tile_base) and the
    remaining triangle is masked multiplicatively with static [128, 512] masks.
  - sa_T [64, 2048] per head is exactly the lhsT the projection needs; the
    3-head projection accumulation is row-tiled (pair + single, DVE combine).

Matmul-operand tiles are declared float32r (byte-identical to fp32); per-stage
knobs bitcast back to float32 where exact fp32 matmuls are wanted (fp32 runs at
1/4 PE rate, fp32r at full rate for free-dim >= 256 but rounds operands to
reduced precision on HW).
"""

import numpy as np

import concourse.bass as bass
import concourse.mybir as mybir
import concourse.tile as tile
from concourse import bacc
from concourse.alu_op_type import AluOpType
from concourse.bass_utils import run_bass_kernel_spmd

F32 = mybir.dt.float32
F32R = mybir.dt.float32r
AF = mybir.ActivationFunctionType

B, N, D = 2, 2048, 768
H, HD = 12, 64
HEADS_PER_CORE = 3
N_CORES = 8
NT = N // 128          # 16 token tiles of 128
NS = N // 512          # 4 query spans of 512
DC = D // 128          # 6 contraction chunks of 128

# Per-stage matmul dtype: True -> exact fp32 (4 cycles/row), False -> fp32r
# (full rate at free-dim >= 256, reduced precision).
STAGE_F32 = {
    "qk": False,   # kqv q/k projection
    "st": False,   # scores S_T
    "pv": False,   # P^T @ v (+ denominator)
    "v": False,    # v projection
    "pr": False,   # output projection
}

# feature flags; row-tiling only pays off when the stage runs exact fp32
# (4 cyc/row) -- in fp32r mode the extra DVE copies cost more than the PE
# savings. None -> derived from STAGE_F32 at build time.
FLAGS = {"rt_st": None, "trim": True, "rt_pr": None, "gp_rb": True, "gp_mask": False}

_CACHE = {}


def _op(ap, stage):
    return ap.bitcast(F32) if STAGE_F32[stage] else ap


def _dt(stage):
    # dtype for compute-produced tiles: fp32r tiles round values at write time,
    # so only use F32R when the consuming matmul stage runs in fp32r.
    return F32 if STAGE_F32[stage] else F32R


def _ddt(*stages):
    # dtype for DMA-fed tiles/DRAM tensors: F32 unless some consumer is fp32r.
    return F32 if all(STAGE_F32[s] for s in stages) else F32R


def build():
    if FLAGS["rt_st"] is None:
        FLAGS["rt_st"] = STAGE_F32["st"]
    if FLAGS["rt_pr"] is None:
        FLAGS["rt_pr"] = STAGE_F32["pr"]
    nc = bacc.Bacc("TRN2", target_bir_lowering=False, debug=False)

    xT_d = nc.dram_tensor("xT", [D, N], _ddt("qk", "v"), kind="ExternalInput").ap()
    wqk_d = nc.dram_tensor("wqk", [128, HEADS_PER_CORE, DC, 128], _ddt("qk"), kind="ExternalInput").ap()
    wv_d = nc.dram_tensor("wv", [128, DC, 256], _ddt("v"), kind="ExternalInput").ap()
    wp_d = nc.dram_tensor("wp", [64, HEADS_PER_CORE, D], _ddt("pr"), kind="ExternalInput").ap()
    bqk_d = nc.dram_tensor("bqk", [128, HEADS_PER_CORE], F32, kind="ExternalInput").ap()
    bvb_d = nc.dram_tensor("bvb", [128, 192], F32, kind="ExternalInput").ap()
    mask_d = nc.dram_tensor("masks", [128, 4, 512], F32, kind="ExternalInput").ap()
    out_d = nc.dram_tensor("out", [N, D], F32, kind="ExternalOutput").ap()

    vw = 192 if STAGE_F32["v"] else 256   # fp32r wants free-dim >= 256

    with tile.TileContext(nc) as tc, \
         nc.allow_low_precision(reason="fp32r matmul operands; accumulation stays fp32"):
        with tc.tile_pool(name="cn", bufs=1) as cn, \
             tc.tile_pool(name="qk", bufs=2) as qkp, \
             tc.tile_pool(name="pt", bufs=6) as ptp, \
             tc.tile_pool(name="sm", bufs=2) as smp, \
             tc.tile_pool(name="ot", bufs=8) as otp, \
             tc.tile_pool(name="psS", bufs=4, space="PSUM") as psS, \
             tc.tile_pool(name="psP", bufs=2, space="PSUM") as psP, \
             tc.tile_pool(name="psM", bufs=2, space="PSUM") as psM:

            # ---- constant loads ----
            # head-0 qk proj needs x span 0 + its wqk slice first; spread DMAs
            # over both HWDGE queues (SP + ACT) so they land in parallel.
            x_sp = []
            xr = xT_d.rearrange("(c p) n -> p c n", p=128)
            wqk_sb = cn.tile([128, HEADS_PER_CORE, DC, 128], _ddt("qk"), name="wqk_sb")
            bqk_sb = cn.tile([128, HEADS_PER_CORE], F32, name="bqk_sb")
            wv_sb = cn.tile([128, DC, 256], _ddt("v"), name="wv_sb")
            bvb_sb = cn.tile([128, 192], F32, name="bvb_sb")
            wp_sb = cn.tile([64, HEADS_PER_CORE, D], _ddt("pr"), name="wp_sb")
            nc.scalar.dma_start(wqk_sb[:, 0, :, :], wqk_d[:, 0, :, :])
            nc.scalar.dma_start(bqk_sb[:], bqk_d)
            for s in range(NS):
                xs = cn.tile([128, DC, 512], _ddt("qk", "v"), name=f"x_sp{s}")
                # halves of the d-chunks on separate queues so they land in parallel
                xsl = xr[:, :, s * 512:(s + 1) * 512]
                nc.sync.dma_start(xs[:, 0:3, :], xsl[:, 0:3, :])
                nc.scalar.dma_start(xs[:, 3:6, :], xsl[:, 3:6, :])
                x_sp.append(xs)
                if s == 0:
                    nc.scalar.dma_start(wv_sb[:], wv_d)
                    nc.sync.dma_start(bvb_sb[:], bvb_d)
                if s == 1:
                    nc.sync.dma_start(wqk_sb[:, 1:3, :, :], wqk_d[:, 1:3, :, :])
            nc.scalar.dma_start(wp_sb[:], wp_d)
            if not FLAGS["gp_mask"]:
                mask_sb = cn.tile([128, 4, 512], F32, name="mask_sb")
                nc.sync.dma_start(mask_sb[:], mask_d)

            vf = cn.tile([128, NT, HEADS_PER_CORE, 65], _dt("pv"), name="vf")
            saT = cn.tile([64, HEADS_PER_CORE, N], _dt("pr"), name="saT")

            def emit_qk(j):
                # q_T/k_T (duplicated into both partition halves when row tiling)
                qh = 128 if FLAGS["rt_st"] else 64
                qt = qkp.tile([qh, N], _dt("st"), name="qt")
                kt = qkp.tile([qh, N], _dt("st"), name="kt")
                for s in range(NS):
                    qk_ps = psM.tile([128, 512], F32, name="qk_ps", tag="misc")
                    for c in range(DC):
                        nc.tensor.matmul(
                            qk_ps[:],
                            _op(wqk_sb[:, j, c, :], "qk"),
                            _op(x_sp[s][:, c, :], "qk"),
                            start=(c == 0), stop=(c == DC - 1),
                        )
                    sl = slice(s * 512, (s + 1) * 512)
                    nc.vector.tensor_scalar_add(qt[0:64, sl], qk_ps[0:64, :],
                                                bqk_sb[0:64, j:j + 1])
                    nc.vector.tensor_scalar_add(kt[0:64, sl], qk_ps[64:128, :],
                                                bqk_sb[64:128, j:j + 1])
                    if FLAGS["rt_st"]:
                        nc.vector.tensor_scalar_add(qt[64:128, sl], qk_ps[0:64, :],
                                                    bqk_sb[0:64, j:j + 1])
                        nc.vector.tensor_scalar_add(kt[64:128, sl], qk_ps[64:128, :],
                                                    bqk_sb[64:128, j:j + 1])
                return qt, kt

            def emit_v(nt_range):
                # V projection for all 3 heads fused: v[n, o], o in [0, 192)
                for nt in nt_range:
                    v_ps = psM.tile([128, 512], F32, name="v_ps", tag="misc")
                    for c in range(DC):
                        nc.tensor.matmul(
                            v_ps[:, 0:vw],
                            _op(x_sp[nt // 4][:, c, (nt % 4) * 128:(nt % 4 + 1) * 128], "v"),
                            _op(wv_sb[:, c, 0:vw], "v"),
                            start=(c == 0), stop=(c == DC - 1),
                        )
                    nc.vector.tensor_tensor(
                        vf[:, nt, :, 0:64],
                        v_ps[:, 0:192].rearrange("p (h d) -> p h d", h=3),
                        bvb_sb[:].rearrange("p (h d) -> p h d", h=3),
                        op=mybir.AluOpType.add,
                    )
                    nc.vector.memset(vf[:, nt, :, 64:65].bitcast(F32), 1.0)

            def emit_attn(j, qt, kt, s_range):
                for s in s_range:
                    nkt = 4 * s + 4
                    pv_ps = psP.tile([65, 512], F32, name="pv_ps", tag="pv")
                    for kt0 in range(0, nkt, 2):
                        pair = (kt0, kt0 + 1)
                        offs, pts = [], []
                        # S_T for the pair, adjacent on PE, in separate row groups
                        for idx, ktile in enumerate(pair):
                            # diagonal blocks only need columns q >= ktile*128
                            off = max(0, (ktile - 4 * s)) * 128 if FLAGS["trim"] else 0
                            if not STAGE_F32["st"]:
                                off = min(off, 256)
                            offs.append(off)
                            half = idx if FLAGS["rt_st"] else 0
                            lo, hi = (0, 64) if half == 0 else (64, 128)
                            sc_ps = psS.tile([128, 512], F32, name="sc_ps", tag="sc")
                            nc.tensor.matmul(
                                sc_ps[:, off:512],
                                _op(kt[lo:hi, ktile * 128:(ktile + 1) * 128], "st"),
                                _op(qt[lo:hi, s * 512 + off:(s + 1) * 512], "st"),
                                start=True, stop=True,
                                tile_position=(lo, 0),
                            )
                            pt = ptp.tile([128, 512], _dt("pv"), name="pt")
                            nc.scalar.activation(pt[:, off:512], sc_ps[:, off:512],
                                                 AF.Exp, scale=0.125)
                            if ktile >= 4 * s:
                                jj = ktile - 4 * s
                                if FLAGS["gp_mask"]:
                                    nc.gpsimd.affine_select(
                                        out=pt[:, off:512],
                                        in_=pt[:, off:512],
                                        compare_op=AluOpType.is_ge, fill=0.0,
                                        base=off - jj * 128,
                                        pattern=[[1, 512 - off]],
                                        channel_multiplier=-1,
                                    )
                                else:
                                    nc.vector.tensor_mul(pt[:, off:512], pt[:, off:512],
                                                         mask_sb[:, jj, off:512])
                            pts.append(pt)
                        for idx, ktile in enumerate(pair):
                            off = offs[idx]
                            nc.tensor.matmul(
                                pv_ps[:, off:512],
                                _op(vf[:, ktile, j, :], "pv"),
                                _op(pts[idx][:, off:512], "pv"),
                                start=(ktile == 0), stop=(ktile == nkt - 1),
                            )
                    rc = smp.tile([1, 512], F32, name="rc")
                    nc.vector.reciprocal(rc[:], pv_ps[64:65, :])
                    rb = smp.tile([64, 512], F32, name="rb")
                    if FLAGS["gp_rb"]:
                        nc.gpsimd.partition_broadcast(rb[:], rc[:])
                    else:
                        ones_t = smp.tile([1, 64], F32, name="ones_t")
                        nc.vector.memset(ones_t[:], 1.0)
                        rb_ps = psM.tile([128, 512], F32, name="rb_ps", tag="misc")
                        nc.tensor.matmul(rb_ps[0:64, :], ones_t[:], rc[:],
                                         start=True, stop=True)
                        nc.vector.tensor_copy(rb[:], rb_ps[0:64, :])
                    nc.vector.tensor_mul(saT[:, j, s * 512:(s + 1) * 512],
                                         pv_ps[0:64, :], rb[:])

            def emit_proj(nt_range):
                for nt in nt_range:
                    ntl = slice(nt * 128, (nt + 1) * 128)
                    for e2 in range(2):
                        esl = slice(e2 * 384, (e2 + 1) * 384)
                        pA = psM.tile([128, 512], F32, name="pA", tag="misc")
                        for j in range(HEADS_PER_CORE):
                            nc.tensor.matmul(
                                pA[:, 0:384],
                                _op(saT[:, j, ntl], "pr"),
                                _op(wp_sb[:, j, esl], "pr"),
                                start=(j == 0), stop=(j == HEADS_PER_CORE - 1),
                            )
                        ot = otp.tile([128, 384], F32, name="ot")
                        nc.vector.tensor_copy(ot[:], pA[:, 0:384])
                        nc.sync.dma_start(out_d[ntl, esl], ot[:])

            if not FLAGS["rt_pr"]:
                qt0, kt0 = emit_qk(0)
                for s in range(NS):
                    emit_v(range(4 * s, 4 * s + 4))
                    emit_attn(0, qt0, kt0, [s])
                qt1, kt1 = emit_qk(1)
                emit_attn(1, qt1, kt1, range(NS))
                qt2, kt2 = emit_qk(2)
                for s in range(NS):
                    emit_attn(2, qt2, kt2, [s])
                    emit_proj(range(4 * s, 4 * s + 4))
                emitted_proj = True
            else:
                qt0, kt0 = emit_qk(0)
                for s in range(NS):
                    emit_v(range(4 * s, 4 * s + 4))
                    emit_attn(0, qt0, kt0, [s])
                for j in range(1, HEADS_PER_CORE):
                    qtj, ktj = emit_qk(j)
                    emit_attn(j, qtj, ktj, range(NS))
                emitted_proj = False

            # ---- output projection (row-tiled variant; spans-outer path did it) ----
            # wp halves: head 0 at partitions 0:64, head 1 at 64:128; head 2 at 0:64 (second slot)
            if FLAGS["rt_pr"]:
                wp2 = cn.tile([128, 2, D], _dt("pr"), name="wp2")
                nc.vector.tensor_copy(_op(wp2[0:64, 0, :], "pr"), _op(wp_sb[:, 0, :], "pr"))
                nc.vector.tensor_copy(_op(wp2[64:128, 0, :], "pr"), _op(wp_sb[:, 1, :], "pr"))
                nc.vector.tensor_copy(_op(wp2[0:64, 1, :], "pr"), _op(wp_sb[:, 2, :], "pr"))
                sa2 = cn.tile([128, 2, N], _dt("pr"), name="sa2")
                nc.vector.tensor_copy(_op(sa2[0:64, 0, :], "pr"), _op(saT[:, 0, :], "pr"))
                nc.vector.tensor_copy(_op(sa2[64:128, 0, :], "pr"), _op(saT[:, 1, :], "pr"))
                nc.vector.tensor_copy(_op(sa2[0:64, 1, :], "pr"), _op(saT[:, 2, :], "pr"))

            for nt in (range(NT) if not emitted_proj else ()):
                ntl = slice(nt * 128, (nt + 1) * 128)
                for e2 in range(2):
                    esl = slice(e2 * 384, (e2 + 1) * 384)
                    if FLAGS["rt_pr"]:
                        pA = psM.tile([128, 512], F32, name="pA", tag="misc")
                        pB = psM.tile([128, 512], F32, name="pB", tag="misc")
                        nc.tensor.matmul(pA[:, 0:384], _op(sa2[0:64, 0, ntl], "pr"),
                                         _op(wp2[0:64, 0, esl], "pr"),
                                         start=True, stop=False, tile_position=(0, 0))
                        nc.tensor.matmul(pB[:, 0:384], _op(sa2[64:128, 0, ntl], "pr"),
                                         _op(wp2[64:128, 0, esl], "pr"),
                                         start=True, stop=True, tile_position=(64, 0))
                        nc.tensor.matmul(pA[:, 0:384], _op(sa2[0:64, 1, ntl], "pr"),
                                         _op(wp2[0:64, 1, esl], "pr"),
                                         start=False, stop=True, tile_position=(0, 0))
                        tmp = otp.tile([128, 384], F32, name="tmp")
                        nc.vector.tensor_copy(tmp[:], pB[:, 0:384])
                        ot = otp.tile([128, 384], F32, name="ot")
                        nc.vector.tensor_tensor(ot[:], pA[:, 0:384], tmp[:],
                                                op=mybir.AluOpType.add)
                    else:
                        pA = psM.tile([128, 512], F32, name="pA", tag="misc")
                        for j in range(HEADS_PER_CORE):
                            nc.tensor.matmul(
                                pA[:, 0:384],
                                _op(saT[:, j, ntl], "pr"),
                                _op(wp_sb[:, j, esl], "pr"),
                                start=(j == 0), stop=(j == HEADS_PER_CORE - 1),
                            )
                        ot = otp.tile([128, 384], F32, name="ot")
                        nc.vector.tensor_copy(ot[:], pA[:, 0:384])
                    nc.sync.dma_start(out_d[ntl, esl], ot[:])

    nc.compile()
    return nc


def _host_shard(x, W_kqv, b_kqv, W_proj, b_proj):
    """Build the 8 per-core input maps."""
    masks = np.zeros((128, 4, 512), dtype=np.float32)
    yy = np.arange(512)[None, :]
    xx = np.arange(128)[:, None]
    for jj in range(4):
        masks[:, jj, :] = (yy >= xx + jj * 128).astype(np.float32)

    in_maps = []
    for c in range(N_CORES):
        b = c // 4
        h0 = (c % 4) * HEADS_PER_CORE
        hs = [h0, h0 + 1, h0 + 2]
        xT = np.ascontiguousarray(x[b].T)                       # [768, 2048]

        wqk = np.empty((128, HEADS_PER_CORE, DC, 128), dtype=np.float32)
        bqk = np.empty((128, HEADS_PER_CORE), dtype=np.float32)
        for j, h in enumerate(hs):
            wj = np.concatenate([W_kqv[h, 64:128], W_kqv[h, 0:64]], axis=0)  # [128, 768]
            # wqk[p, j, c, m] = wj[m, c*128+p]
            wqk[:, j, :, :] = wj.T.reshape(DC, 128, 128).transpose(1, 0, 2)
            bqk[:, j] = np.concatenate([b_kqv[h, 64:128], b_kqv[h, 0:64]])

        wv_all = np.zeros((D, 256), dtype=np.float32)
        for j, h in enumerate(hs):
            wv_all[:, j * 64:(j + 1) * 64] = W_kqv[h, 128:192].T
        wv = np.ascontiguousarray(wv_all.reshape(DC, 128, 256).transpose(1, 0, 2))

        wp = np.empty((64, HEADS_PER_CORE, D), dtype=np.float32)
        for j, h in enumerate(hs):
            wp[:, j, :] = W_proj[:, h * 64:(h + 1) * 64].T

        bvb = np.tile(np.concatenate([b_kqv[h, 128:192] for h in hs])[None, :],
                      (128, 1)).astype(np.float32)

        in_maps.append({
            "xT": xT, "wqk": wqk, "wv": wv, "wp": wp,
            "bqk": bqk, "bvb": bvb, "masks": masks,
        })
    return in_maps


def kernel(x, W_kqv, b_kqv, W_proj, b_proj):
    x = np.asarray(x, dtype=np.float32)
    W_kqv = np.asarray(W_kqv, dtype=np.float32)
    b_kqv = np.asarray(b_kqv, dtype=np.float32)
    W_proj = np.asarray(W_proj, dtype=np.float32)
    b_proj = np.asarray(b_proj, dtype=np.float32)

    if "nc" not in _CACHE:
        _CACHE["nc"] = build()
    nc = _CACHE["nc"]

    in_maps = _host_shard(x, W_kqv, b_kqv, W_proj, b_proj)
    res = run_bass_kernel_spmd(nc, in_maps, list(range(N_CORES)))

    out = np.empty((B, N, D), dtype=np.float32)
    for b in range(B):
        acc = res.results[4 * b]["out"].astype(np.float32)
        for c in range(4 * b + 1, 4 * b + 4):
            acc = acc + res.results[c]["out"]
        out[b] = acc + b_proj[None, :]
    return out
